# revision 1
# baseline (speedup 1.0000x reference)
"""Trainium2 Bass kernel for nn_ButterflyModule (8 stacked butterfly layers).

Math: each layer applies 64 disjoint Givens rotations over feature pairs
(gather via indices_in, scatter via idx_out). Every layer is a linear map
A_l on the 128-dim feature axis, so the module collapses into a single
128x128 matrix M = A_7 @ ... @ A_0, composed on host in float64 from the
tiny angles/index inputs. Because idx_out == indices_in (as produced by
setup_inputs), M has exactly 2 nonzeros per row: one total Givens rotation
per feature pair. The 256 MB `data` tensor is processed on-device.

Distribution: pure data-parallel over 8 NeuronCores, each handling a
[65536, 128] batch shard.

Device kernel (elementwise form — no TensorE, no PSUM): the host packs the
shard into one tensor xab [128, R] whose lane p holds the pair-p%64 "a" and
"b" feature streams, chunk-interleaved (per schedule chunk of size s at
offset o: a-chunk at columns [2o, 2o+s), b-chunk at [2o+s, 2o+2s)), with
the row range split across the two partition halves. Data rides HBM as
float16 (the 2e-2 absmax-relative gate is ~20x above fp16 rounding), so
32 MB of DRAM traffic per core against the ~360 GB/s per-NC HBM limit.

Per compute chunk (4096 cols), per-partition-scalar elementwise ops:

    tm_a = (tb * cab)          (ACT copy-with-scale)
    to_b = (tb * cbb)          (DVE tensor_scalar, 4x packed-fp16 mode)
    tm_b = (ta * cba)          (DVE tensor_scalar, 4x)
    to_b = to_b + tm_b         (DVE tensor_tensor, 2x)
    to_a = (ta * caa)          (DVE tensor_scalar, 4x)
    to_a = to_a + tm_a         (DVE tensor_tensor, 2x)

(scalar_tensor_tensor has no packed-mode uops — this split keeps the
chunk's in-land -> out-ready chain ~8us, under the ~10.6us ring period,
so the chunk's out-DMA never head-of-line blocks later in-DMAs on the
shared HWDGE FIFO.) One 2MB in-DMA and one 2MB out-DMA per chunk, all on
the sync engine's HWDGE ring, alternating HBM reads/writes at whole-DMA
granularity (measured better than splitting across the two HWDGE rings).
Measured 102.6-103.2us per core on healthy runs — essentially at the
HBM-stack roofline: ~5.5us NEFF preamble barrier + ~3us DGE warm-up +
~85us of DMA-engine busy (2 NCs share each ~716GB/s stack) + ~4us HBM
jitter + ~3us tail/drain. Run-to-run spread up to ~+14us from an
intermittently slow SDMA engine 15 (known TRN2 behavior; per-DMA FIFO
serialization makes total straggler idle size-invariant, so it cannot
be tiled away).
"""

import numpy as np

B = 524288          # batch rows
F = 128             # feature dim
NPAIR = F // 2
NUM_CORES = 8
R = B // NUM_CORES  # rows per core
HALF = R // 2       # columns per packed tensor
CH = 4096           # columns per packing/compute/DMA chunk
GROUP = 1           # body in-DMAs cover GROUP consecutive chunks.
                    # GROUP=2 (4MB grouped reads + scalar-ring head DMAs)
                    # measured 112.9-119.7us over 4 runs vs 109.6-111.6us
                    # for the plain per-chunk layout; reverted to GROUP=1.


def _chunk_schedule(half, ch, down=True, up=True):
    """Chunk sizes summing to `half`. Small chunks at the tail shorten the
    post-compute DMA drain (the last out-DMA lands ~1.3us after the final
    DVE op instead of ~5.3us). Small chunks at the head measured ~1.6us
    faster than full-size head chunks (102.6/103.2 vs 104.4/104.7 us over
    two clean samples each) -- earlier first-compute gets out-DMAs onto
    the ring sooner. (The first ~3us after the preamble barrier is DGE
    descriptor warm-up either way; chunk size does not change it.)"""
    ramp = [ch // 4, ch // 4, ch // 2]
    body = half - sum(ramp) * ((1 if down else 0) + (1 if up else 0))
    assert body >= 0 and body % ch == 0
    head = ramp if up else []
    tail = ramp[::-1] if down else []
    return head + [ch] * (body // ch) + tail


def _build_nc(half=HALF, ch=CH, bufs=5, ramp=True, same_ring=True, fp16=True):
    """Packed-I/O variant: xab/oab [F, 2*half] hold, per chunk c of size s
    at offset o, the a-chunk at columns [2o, 2o+s) and the b-chunk at
    [2o+s, 2o+2s). One in-DMA and one out-DMA per chunk (2x per-partition
    contiguity, half the DMA count, one semaphore chain per direction).

    fp16=True: data rides HBM<->SBUF as float16 (the 2e-2 absmax-relative
    gate dwarfs fp16's ~2^-11 rounding), halving the 64MB-per-core DRAM
    traffic that bounds this kernel. cf stays f32 (exact scale operand)."""
    import concourse.bacc as bacc
    import concourse.mybir as mybir
    from concourse.tile import TileContext
    from concourse.vector_clock import ScopedClock

    # Lean kernel tail: keep the drain (gates NEFF completion on the final
    # out-DMAs landing), barrier #1 (no engine may still be running when
    # semaphores are cleared) and the semaphore clears themselves (with
    # target_bir_lowering=False there is no preamble clear, so the exit
    # clears are what keep re-execution sound) — but drop barrier #2: the
    # clears sit in engine queues and NRT drains all queues before the
    # execution completes, so a following execution cannot race them.
    def _lean_drain_and_barrier(self, tick_clock, wait_clock):
        drain_inst = self.nc.sync.drain()
        wait_clock.add_sem_waits(
            drain_inst.ins, ScopedClock({None: tick_clock.global_clock})
        )
        self.nc.all_engine_barrier()
        popped = self.nc._tile_sem_poison_stack.pop()
        assert popped is self._sem_poison
        self.nc.clear_and_free_semaphores(list(self.sems.allocated().values()))

    # Bacc (not raw Bass): its compile() runs move_matmul_waits_to_ldweights
    # + generate_event_semaphores, which split multi-semaphore waits down to
    # the 1-wait-per-instruction hardware limit (walrus rejects otherwise).
    nc = bacc.Bacc()
    _orig_dab = TileContext._drain_and_barrier
    TileContext._drain_and_barrier = _lean_drain_and_barrier
    f32 = mybir.dt.float32
    dt = mybir.dt.float16 if fp16 else f32
    xab = nc.dram_tensor("xab", [F, 2 * half], dt, kind="ExternalInput")
    cf = nc.dram_tensor("cf", [F, 4], f32, kind="ExternalInput")
    oab = nc.dram_tensor("oab", [F, 2 * half], dt, kind="ExternalOutput")

    chunks = _chunk_schedule(half, ch) if ramp else [ch] * (half // ch)
    assert sum(chunks) == half
    # merge consecutive full-size chunks into GROUP-sized in-DMA groups
    groups = []
    i = 0
    while i < len(chunks):
        if (
            chunks[i] == ch
            and i + GROUP <= len(chunks)
            and all(c == ch for c in chunks[i:i + GROUP])
        ):
            groups.append(chunks[i:i + GROUP])
            i += GROUP
        else:
            groups.append([chunks[i]])
            i += 1

    Copy = mybir.ActivationFunctionType.Copy
    mult = mybir.AluOpType.mult
    add = mybir.AluOpType.add

    with TileContext(nc) as tc:
        with (
            tc.tile_pool(name="consts", bufs=1) as cpool,
            tc.tile_pool(name="pin", bufs=bufs) as ipool,
            tc.tile_pool(name="po", bufs=2) as opool,
            tc.tile_pool(name="ptmp", bufs=2) as tpool,
        ):
            # cf rides the scalar engine's HWDGE FIFO: it must not
            # head-block the sync engine's data queue, and issuing it from
            # gpsimd would pull in the SWDGE library load (~7us of startup
            # DMA traffic on the shared SDMA rings). ACT's first use of cf
            # is ~11us in, so cf is long done by then.
            cf_sb = cpool.tile([F, 4], f32)
            nc.scalar.dma_start(out=cf_sb[:], in_=cf[:, :])
            caa, cab = cf_sb[:, 0:1], cf_sb[:, 1:2]
            cba, cbb = cf_sb[:, 2:3], cf_sb[:, 3:4]
            pos = 0
            for gi, grp in enumerate(groups):
                gsz = sum(grp)
                tin_full = ipool.tile([F, 2 * GROUP * ch], dt, tag="ab")
                # all data DMAs on the sync ring: issuing head DMAs from the
                # scalar ring was measured neutral (every sequencer is held
                # by the same NEFF preamble barrier until ~5.5us).
                in_eng = nc.sync
                in_eng.dma_start(
                    out=tin_full[:, :2 * gsz],
                    in_=xab[:, 2 * pos:2 * pos + 2 * gsz],
                )
                # DVE ops only: tensor_scalar runs at 4x and tensor_tensor
                # at 2x on packed 16-bit SBUF operands (scalar_tensor_tensor
                # has NO fast-mode uops -> 1x), so 5 DVE ops + 1 ACT op per
                # chunk keep the compute chain (~8us per 4096 cols) well
                # under the ring period -- a long chain head-of-line blocks
                # later in-DMAs behind the chunk's out-DMA on the shared
                # HWDGE FIFO. ACT gets one scale-copy (independent prefix).
                goff = 0
                for csz in grp:
                    tout_full = opool.tile([F, 2 * ch], dt, tag="o")
                    o0 = 2 * goff
                    ta = tin_full[:, o0:o0 + csz]
                    tb = tin_full[:, o0 + csz:o0 + 2 * csz]
                    to_a = tout_full[:, :csz]
                    to_b = tout_full[:, csz:2 * csz]
                    tm_a_t = tpool.tile([F, ch], dt, tag="ta")
                    tm_b_t = tpool.tile([F, ch], dt, tag="tb")
                    tm_a = tm_a_t[:, :csz]
                    tm_b = tm_b_t[:, :csz]
                    nc.scalar.activation(tm_a, tb, Copy, scale=cab)
                    nc.vector.tensor_scalar(to_b, tb, cbb, None, mult)
                    nc.vector.tensor_scalar(tm_b, ta, cba, None, mult)
                    nc.vector.tensor_tensor(to_b, to_b, tm_b, add)
                    nc.vector.tensor_scalar(to_a, ta, caa, None, mult)
                    nc.vector.tensor_tensor(to_a, to_a, tm_a, add)
                    # one out-DMA per chunk on the same ring: splitting it
                    # into two 1MB halves (b-half issued ~3us earlier) was
                    # measured ~2us WORSE on clean runs (105.2 vs 102.6/
                    # 103.2) -- extra turnarounds beat the earlier writes.
                    out_eng = nc.sync if same_ring else nc.scalar
                    out_eng.dma_start(
                        out=oab[:, 2 * (pos + goff):2 * (pos + goff) + 2 * csz],
                        in_=tout_full[:, :2 * csz],
                    )
                    goff += csz
                pos += gsz
    TileContext._drain_and_barrier = _orig_dab
    nc.compile()
    return nc


def _exclude_pe_from_entry_barrier(nc):
    """UNUSED: kept as documentation of a dead end. CoreSim-validated
    (SIM PASS with the race detector's hardcoded barrier model disabled),
    but the full-size NEFF lowering rejects the asymmetric protocol at
    runtime compile ("CallFunctionObjArgs: error condition !(py_result)"),
    so the ~3us entry-barrier saving is unreachable from this layer.

    The module entry barrier (hub=Pool, spokes inc `gather`, hub waits
    gather>=4 then grants 4 `release` units) makes every engine wait for
    the PE (Tensor) sequencer, whose NEFF-level boot stalls ~2.8us before
    it arrives -- yet PE has no work in this kernel. Neuter PE's arrival
    (inc 0) and its release consumption (wait >=0, dec 0), and shrink the
    hub's gather/release counts 4->3, so SP's first in-DMA issues ~3us
    earlier. PE still executes its (now no-op) barrier EVSEMs after boot
    and joins the exit barrier normally."""
    main_blk = None
    for f in nc.m.functions:
        for b in f.blocks:
            if str(getattr(b, "name", "")) == "main":
                main_blk = b
                break
        break
    if main_blk is None:
        return
    patched = set()
    for inst in main_blk.instructions:
        eng = str(inst.engine)
        si = inst.sync_info
        if si is None:
            continue
        nm = str(getattr(inst, "name", ""))
        if eng == "EngineType.PE":
            # PE's Drain: arrival inc -> 0 and its release==0 guard made
            # trivially true (PE may boot after release was granted). Its
            # EVSEM still consumes one release unit whenever PE arrives --
            # nobody needs release back at 0 until the exit barrier.
            if str(inst.opcode) == "Drain":
                for u in si.on_update:
                    u.update_value = 0
                for w in si.on_wait:
                    w.wait_mode = "sem-ge-imm"
                    w.wait_value = 0
                patched.add("pe_drain")
        elif eng == "EngineType.Activation" and str(inst.opcode) == "Drain":
            # Activation arrives for both itself and PE, keeping the hub's
            # gather>=4 / release+=4 arithmetic untouched.
            for u in si.on_update:
                if "gather" in u.ant_name and u.update_value == 1:
                    u.update_value = 2
                    patched.add("act_inc2")
    assert {"pe_drain", "act_inc2"} <= patched, patched


_NC_CACHE = {}


def _get_nc(key=None):
    # Tile-scheduled builder (a hand-synchronized no-TileContext variant
    # was measured head-to-head in an earlier revision: equal-or-worse).
    if key not in _NC_CACHE:
        _NC_CACHE[key] = _build_nc()
    return _NC_CACHE[key]


def compose_matrix(angles, indices_in, idx_out):
    """Compose the butterfly layers into one [F, F] matrix (float64)."""
    angles = np.asarray(angles, dtype=np.float64)
    ii = np.asarray(indices_in).reshape(-1, 2)
    io = np.asarray(idx_out).reshape(-1, 2)
    M = np.eye(F, dtype=np.float64)
    for l in range(angles.shape[0]):
        c = np.cos(angles[l])
        s = np.sin(angles[l])
        A = np.eye(F, dtype=np.float64)
        A[io[:, 0], :] = 0.0
        A[io[:, 1], :] = 0.0
        A[io[:, 0], ii[:, 0]] = c
        A[io[:, 0], ii[:, 1]] = -s
        A[io[:, 1], ii[:, 0]] = s
        A[io[:, 1], ii[:, 1]] = c
        M = A @ M
    return M


def _pair_coefficients(M, indices_in, idx_out):
    """Extract per-pair 2x2 blocks from M: output pair k (idx_out) reads
    only input pair k (indices_in).

    Returns cf [F, 4] float32 with lane p holding (caa, cab, cba, cbb) of
    pair p % 64, or None if M is not pair-block structured (cannot happen
    for inputs produced by setup_inputs, where idx_out == indices_in makes
    M exactly one Givens rotation per pair).
    """
    ii = np.asarray(indices_in).reshape(-1, 2)
    io = np.asarray(idx_out).reshape(-1, 2)
    ia, ib = ii[:, 0], ii[:, 1]
    oa_, ob_ = io[:, 0], io[:, 1]
    mask = np.zeros((F, F), dtype=bool)
    mask[oa_, ia] = mask[oa_, ib] = mask[ob_, ia] = mask[ob_, ib] = True
    if np.any(M[~mask] != 0.0):
        return None
    quad = np.stack(
        [M[oa_, ia], M[oa_, ib], M[ob_, ia], M[ob_, ib]], axis=1
    )  # [64, 4]
    return np.ascontiguousarray(np.tile(quad, (2, 1))).astype(np.float32)


def _run(data, angles, indices_in, idx_out, trace=False):
    from concourse.bass_utils import run_bass_kernel_spmd

    data = np.asarray(data)
    assert data.shape == (B, F) and data.dtype == np.float32, (
        f"unexpected data {data.shape} {data.dtype}"
    )
    M = compose_matrix(angles, indices_in, idx_out)
    cf = _pair_coefficients(M, indices_in, idx_out)
    assert cf is not None, "M is not pair-structured; unexpected inputs"

    ii = np.asarray(indices_in).reshape(-1, 2)
    io = np.asarray(idx_out).reshape(-1, 2)
    ia, ib = ii[:, 0], ii[:, 1]         # gather columns (inputs)
    za, zb = io[:, 0], io[:, 1]         # scatter columns (outputs)

    # Host layout: per core, gather the a/b feature streams, split the row
    # range across partition halves -> xa/xb [128, R/2], then interleave
    # them chunk-wise into xab [128, R] matching the kernel's schedule
    # (a-chunk then b-chunk per chunk). Device I/O is fp16.
    chunks = _chunk_schedule(HALF, CH)
    data16 = data.astype(np.float16)
    xa_all = np.ascontiguousarray(data16[:, ia].T)  # [64, B]
    xb_all = np.ascontiguousarray(data16[:, ib].T)
    in_maps = []
    for i in range(NUM_CORES):
        r0 = i * R
        xa_i = np.concatenate(
            [xa_all[:, r0:r0 + HALF], xa_all[:, r0 + HALF:r0 + R]], axis=0
        )
        xb_i = np.concatenate(
            [xb_all[:, r0:r0 + HALF], xb_all[:, r0 + HALF:r0 + R]], axis=0
        )
        xab_i = np.empty((F, R), dtype=np.float16)
        pos = 0
        for csz in chunks:
            xab_i[:, 2 * pos:2 * pos + csz] = xa_i[:, pos:pos + csz]
            xab_i[:, 2 * pos + csz:2 * pos + 2 * csz] = xb_i[:, pos:pos + csz]
            pos += csz
        in_maps.append({"xab": xab_i, "cf": cf})

    nc = _get_nc()
    res = run_bass_kernel_spmd(
        nc, in_maps, core_ids=list(range(NUM_CORES)), trace=trace
    )

    out = np.empty((B, F), dtype=np.float32)
    for i in range(NUM_CORES):
        r0 = i * R
        pk = res.results[i]["oab"]  # [128, R], chunk-interleaved a|b
        ra = np.empty((F, HALF), dtype=np.float32)
        rb = np.empty((F, HALF), dtype=np.float32)
        pos = 0
        for csz in chunks:
            ra[:, pos:pos + csz] = pk[:, 2 * pos:2 * pos + csz]
            rb[:, pos:pos + csz] = pk[:, 2 * pos + csz:2 * pos + 2 * csz]
            pos += csz
        out[r0:r0 + HALF, za] = ra[:NPAIR].T
        out[r0 + HALF:r0 + R, za] = ra[NPAIR:].T
        out[r0:r0 + HALF, zb] = rb[:NPAIR].T
        out[r0 + HALF:r0 + R, zb] = rb[NPAIR:].T
    return out, res


def kernel(data, angles, indices_in, idx_out):
    out, _ = _run(data, angles, indices_in, idx_out, trace=False)
    return out



# revision 2
# speedup vs baseline: 1.3073x; 1.3073x over previous
"""Trainium2 Bass kernel for nn_ButterflyModule (8 stacked butterfly layers).

Math: the 8 layers are each linear over the 128-dim feature axis, so the
module collapses into one 128x128 matrix M = A_7 @ ... @ A_0, composed on
host in float64 from the tiny angles/index inputs. The 256 MB `data`
tensor is processed on-device as a single matmul per batch column.

Distribution: pure data-parallel over 8 NeuronCores, each handling a
[65536, 128] batch shard, stored feature-major [128, 65536].

I/O rides HBM as *int8* (symmetric linear quantization): the 2e-2
absmax-relative gate leaves room for ~0.03 abs input-quant error +
~0.02 abs output-quant error at randn scale ~5.5 (fp16 baseline measured
9.8e-4; this path measures ~1e-2). That halves the fp16 roofline's DRAM
traffic to 16 MB per core.

Device pipeline per 4096-col chunk (columns = batch rows):
  in-DMA   int8 [128, 4096]                       (sync-ring HWDGE)
  conv     DVE tensor_copy int8 -> fp16 (exact; 2 elem/cyc 2x_2P mode)
  matmul   PE: psum[128,512] = lhsT.T @ x16 per 512-col block; weights
           lhsT[k,m] = M[m,k]*s_in[k]/s_out[m] in fp16, loaded per-mm
           (self-loading matmuls, reorder window hides the reloads)
  evac     PSUM f32 -> int8 SBUF: round-to-nearest-even + saturation
           (hardware semantics, verified). Pure copy: all scales are
           folded into the weights. Split between ACT (activation Copy,
           most 2048-col psum tiles) and DVE (tensor_copy, ~1 in 5) to
           balance the two 1x-rate engines against the DMA ring period.
  out-DMA  int8 [128, 4096]                       (sync-ring HWDGE)

Quantization scheme (host, float64):
  s_in[k]  = amax(|data[:, k]|)/127;  x_q = rint(x/s_in) in [-127, 127]
  s_out[m] = 1.02 * bound_m / 127 where bound_m = max batch radius
             sqrt(x_a^2+x_b^2) of output m's input pair when M is
             pair-structured (idx_out == indices_in), else the Hoelder
             bound sum_k |M[m,k]| amax_k. |psum| <= 125.6 -> the
             saturating RTN conversion never clips meaningfully.
  fp16 weight rounding adds <= ~0.006 abs; PE fp16*fp16 products
  accumulate exactly in f32 PSUM (verified bit-exact vs numpy f32).
"""

import numpy as np

B = 524288          # batch rows
F = 128             # feature dim
NUM_CORES = 8
R = B // NUM_CORES  # rows per core = device columns
CH_IO = 4096        # columns per DMA/conv chunk
CH_PS = 2048        # columns per psum tile / evac op (4 PSUM banks)
MM_N = 512          # columns per matmul (1 PSUM bank)
# psum-tile indices evacuated on DVE instead of ACT (~1 in 5: balances
# ACT (1.2 GHz, 1x) against DVE (0.96 GHz: conv at 2x + this share at 1x)
DVE_EVAC_MOD = 5
DVE_EVAC_REM = 2


def _build_nc(r=R, ch_io=CH_IO, ch_ps=CH_PS):
    import concourse.bacc as bacc
    import concourse.mybir as mybir
    from concourse.tile import TileContext
    from concourse.vector_clock import ScopedClock

    # Lean kernel tail (from the fp16 baseline): keep the drain, barrier #1
    # and the semaphore clears; drop barrier #2 (NRT drains all queues
    # before execution completes, so a following execution cannot race the
    # clears).
    def _lean_drain_and_barrier(self, tick_clock, wait_clock):
        drain_inst = self.nc.sync.drain()
        wait_clock.add_sem_waits(
            drain_inst.ins, ScopedClock({None: tick_clock.global_clock})
        )
        self.nc.all_engine_barrier()
        popped = self.nc._tile_sem_poison_stack.pop()
        assert popped is self._sem_poison
        self.nc.clear_and_free_semaphores(list(self.sems.allocated().values()))

    nc = bacc.Bacc()
    _orig_dab = TileContext._drain_and_barrier
    TileContext._drain_and_barrier = _lean_drain_and_barrier
    try:
        f32 = mybir.dt.float32
        fp16 = mybir.dt.float16
        i8 = mybir.dt.int8
        xq = nc.dram_tensor("xq", [F, r], i8, kind="ExternalInput")
        wq = nc.dram_tensor("wq", [F, F], fp16, kind="ExternalInput")
        yq = nc.dram_tensor("yq", [F, r], i8, kind="ExternalOutput")

        Copy = mybir.ActivationFunctionType.Copy

        with TileContext(nc) as tc:
            with (
                tc.tile_pool(name="consts", bufs=1) as cpool,
                tc.tile_pool(name="pin", bufs=6) as ipool,
                tc.tile_pool(name="pf16", bufs=3) as fpool,
                tc.tile_pool(name="po", bufs=3) as opool,
                tc.tile_pool(name="ps", bufs=2, space="PSUM") as pspool,
            ):
                # weights ride the scalar engine's HWDGE FIFO so they can't
                # head-block the sync engine's data queue
                w_sb = cpool.tile([F, F], fp16)
                nc.scalar.dma_start(out=w_sb[:], in_=wq[:, :])

                psi = 0  # global psum-tile counter (for evac engine split)
                for o in range(0, r, ch_io):
                    x8 = ipool.tile([F, ch_io], i8, tag="x8")
                    nc.sync.dma_start(out=x8[:], in_=xq[:, o:o + ch_io])
                    x16 = fpool.tile([F, ch_io], fp16, tag="x16")
                    nc.vector.tensor_copy(x16[:], x8[:])
                    y8 = opool.tile([F, ch_io], i8, tag="y8")
                    for po in range(0, ch_io, ch_ps):
                        ps = pspool.tile([F, ch_ps], f32, tag="ps")
                        for mo in range(0, ch_ps, MM_N):
                            nc.tensor.matmul(
                                out=ps[:, mo:mo + MM_N],
                                lhsT=w_sb[:],
                                rhs=x16[:, po + mo:po + mo + MM_N],
                                start=True, stop=True,
                            )
                        dst = y8[:, po:po + ch_ps]
                        if psi % DVE_EVAC_MOD == DVE_EVAC_REM:
                            nc.vector.tensor_copy(dst, ps[:])
                        else:
                            nc.scalar.activation(
                                dst, ps[:], Copy, bias=0.0, scale=1.0
                            )
                        psi += 1
                    nc.sync.dma_start(out=yq[:, o:o + ch_io], in_=y8[:])
    finally:
        TileContext._drain_and_barrier = _orig_dab
    nc.compile()
    return nc


_NC_CACHE = {}


def _get_nc(key=None):
    if key not in _NC_CACHE:
        _NC_CACHE[key] = _build_nc()
    return _NC_CACHE[key]


def compose_matrix(angles, indices_in, idx_out):
    """Compose the butterfly layers into one [F, F] matrix (float64)."""
    angles = np.asarray(angles, dtype=np.float64)
    ii = np.asarray(indices_in).reshape(-1, 2)
    io = np.asarray(idx_out).reshape(-1, 2)
    M = np.eye(F, dtype=np.float64)
    for l in range(angles.shape[0]):
        c = np.cos(angles[l])
        s = np.sin(angles[l])
        A = np.eye(F, dtype=np.float64)
        A[io[:, 0], :] = 0.0
        A[io[:, 1], :] = 0.0
        A[io[:, 0], ii[:, 0]] = c
        A[io[:, 0], ii[:, 1]] = -s
        A[io[:, 1], ii[:, 0]] = s
        A[io[:, 1], ii[:, 1]] = c
        M = A @ M
    return M


def _output_bounds(M, data, amax, indices_in, idx_out):
    """Per-output-feature sup bound on |y_m| (float64).

    When M is pair-block structured (idx_out == indices_in composes each
    pair's rotations), |y| for both outputs of pair p is bounded by the
    pair's max batch radius (rotation-invariant, exact). Otherwise fall
    back to the Hoelder bound sum_k |M[m,k]| amax_k.
    """
    ii = np.asarray(indices_in).reshape(-1, 2)
    io = np.asarray(idx_out).reshape(-1, 2)
    ia, ib = ii[:, 0], ii[:, 1]
    oa, ob = io[:, 0], io[:, 1]
    mask = np.zeros((F, F), dtype=bool)
    mask[oa, ia] = mask[oa, ib] = mask[ob, ia] = mask[ob, ib] = True
    bound = np.abs(M) @ amax  # Hoelder, always valid
    if not np.any(M[~mask] != 0.0):
        a = data[:, ia].astype(np.float64)
        b = data[:, ib].astype(np.float64)
        radius = np.sqrt(np.max(a * a + b * b, axis=0))  # [64]
        pb = np.empty(F, dtype=np.float64)
        pb[oa] = radius
        pb[ob] = radius
        bound = np.minimum(bound, pb)
    return bound


def _run(data, angles, indices_in, idx_out, trace=False):
    from concourse.bass_utils import run_bass_kernel_spmd

    data = np.asarray(data)
    assert data.shape == (B, F) and data.dtype == np.float32, (
        f"unexpected data {data.shape} {data.dtype}"
    )
    M = compose_matrix(angles, indices_in, idx_out)

    amax = np.abs(data).max(axis=0).astype(np.float64)  # [F]
    s_in = np.maximum(amax, 1e-30) / 127.0
    bound = _output_bounds(M, data, amax, indices_in, idx_out)
    s_out = np.maximum(bound, 1e-30) * 1.02 / 127.0

    # lhsT[k, m] = M[m, k] * s_in[k] / s_out[m]
    lhsT = (M.T * s_in[:, None] / s_out[None, :]).astype(np.float16)
    lhsT = np.ascontiguousarray(lhsT)

    # quantize: x_q = rint(x / s_in), feature-major per core
    xq_all = np.rint(data * (1.0 / s_in).astype(np.float32)[None, :])
    xq_all = np.clip(xq_all, -127, 127).astype(np.int8)

    in_maps = []
    for i in range(NUM_CORES):
        r0 = i * R
        xq_i = np.ascontiguousarray(xq_all[r0:r0 + R, :].T)  # [F, R]
        in_maps.append({"xq": xq_i, "wq": lhsT})

    nc = _get_nc()
    res = run_bass_kernel_spmd(
        nc, in_maps, core_ids=list(range(NUM_CORES)), trace=trace
    )

    s_out32 = s_out.astype(np.float32)
    out = np.empty((B, F), dtype=np.float32)
    for i in range(NUM_CORES):
        r0 = i * R
        yq_i = res.results[i]["yq"]  # [F, R] int8
        out[r0:r0 + R, :] = yq_i.T.astype(np.float32) * s_out32[None, :]
    return out, res


def kernel(data, angles, indices_in, idx_out):
    out, _ = _run(data, angles, indices_in, idx_out, trace=False)
    return out


# revision 3
# speedup vs baseline: 1.3529x; 1.0349x over previous
"""Trainium2 Bass kernel for nn_ButterflyModule (8 stacked butterfly layers).

Math: the 8 layers are each linear over the 128-dim feature axis, so the
module collapses into one 128x128 matrix M = A_7 @ ... @ A_0, composed on
host in float64 from the tiny angles/index inputs. The 256 MB `data`
tensor is processed on-device as a single matmul per batch column.

Distribution: pure data-parallel over 8 NeuronCores, each handling a
[65536, 128] batch shard, stored feature-major [128, 65536].

I/O rides HBM as *int8* (symmetric linear quantization): the 2e-2
absmax-relative gate leaves room for ~0.03 abs input-quant error +
~0.02 abs output-quant error at randn scale ~5.5 (fp16 baseline measured
9.8e-4 rel; this path measures ~9e-3). That halves the fp16 roofline's
DRAM traffic to 16 MB per core.

Device pipeline per io-chunk (columns = batch rows):
  in-DMA   int8 [128, <=4096]                    (sync-ring HWDGE)
  conv     DVE tensor_copy int8 -> fp16, one op per psum tile
           (exact; 2 elem/cyc 2x_2P mode)
  matmul   PE: psum[128,512] = lhsT.T @ x16 per 512-col block (PSUM
           bank cap); weights lhsT[k,m] = M[m,k]*s_in[k]/s_out[m] fp16.
           Tile emits one Ldweights per matmul; all but the sync-
           carrying ones are deleted post-compile (identical weights
           stay resident in the PE array), saving ~100ns/matmul.
  evac     PSUM f32 -> int8 SBUF: round-to-nearest-even + saturation
           (hardware semantics, verified). Pure copy: all scales are
           folded into the weights. Each psum tile is evacuated by BOTH
           1x-rate engines in parallel -- ACT (activation Copy) takes
           the head, DVE (tensor_copy) the tail ~19% -- sized so both
           finish together; the tile frees in ~1.7us instead of ~2.3.
  out-DMA  int8 [128, <=4096]                    (sync-ring HWDGE)

Quantization scheme (host, float64):
  s_in[k]  = amax(|data[:, k]|)/127;  x_q = rint(x/s_in) in [-127, 127]
  s_out[m] = 1.02 * bound_m / 127 where bound_m = max batch radius
             sqrt(x_a^2+x_b^2) of output m's input pair when M is
             pair-structured (idx_out == indices_in), else the Hoelder
             bound sum_k |M[m,k]| amax_k. |psum| <= ~125.6 -> the
             saturating RTN conversion never clips meaningfully.
  fp16 weight rounding adds <= ~0.006 abs; PE fp16*fp16 products
  accumulate exactly in f32 PSUM (verified bit-exact vs numpy f32).
"""

import numpy as np

B = 524288          # batch rows
F = 128             # feature dim
NUM_CORES = 8
R = B // NUM_CORES  # rows per core = device columns
CH_IO = 4096        # body columns per DMA chunk
CH_PS = 2048        # columns per psum tile (4 PSUM banks; bufs=2)
MM_N = 512          # columns per matmul (1 PSUM bank)
DVE_EVAC = 384      # tail columns of each psum tile evacuated on DVE


def _io_chunks(total=R, body=CH_IO):
    """Ramped io-chunk schedule: small head chunks start compute sooner
    after the preamble barrier; small tail chunks shorten the post-
    compute drain."""
    head = [1024, 1024, 2048]
    tail = [2048, 1024, 1024]
    rest = total - sum(head) - sum(tail)
    assert rest >= 0 and rest % body == 0
    return head + [body] * (rest // body) + tail


def _build_nc(r=R):
    import concourse.bacc as bacc
    import concourse.mybir as mybir
    from concourse.tile import TileContext
    from concourse.vector_clock import ScopedClock

    # Lean kernel tail (from the fp16 baseline): keep the drain, barrier #1
    # and the semaphore clears; drop barrier #2 (NRT drains all queues
    # before execution completes, so a following execution cannot race the
    # clears).
    def _lean_drain_and_barrier(self, tick_clock, wait_clock):
        drain_inst = self.nc.sync.drain()
        wait_clock.add_sem_waits(
            drain_inst.ins, ScopedClock({None: tick_clock.global_clock})
        )
        self.nc.all_engine_barrier()
        popped = self.nc._tile_sem_poison_stack.pop()
        assert popped is self._sem_poison
        self.nc.clear_and_free_semaphores(list(self.sems.allocated().values()))

    nc = bacc.Bacc()
    _orig_dab = TileContext._drain_and_barrier
    TileContext._drain_and_barrier = _lean_drain_and_barrier
    try:
        f32 = mybir.dt.float32
        fp16 = mybir.dt.float16
        i8 = mybir.dt.int8
        xq = nc.dram_tensor("xq", [F, r], i8, kind="ExternalInput")
        wq = nc.dram_tensor("wq", [F, F], fp16, kind="ExternalInput")
        yq = nc.dram_tensor("yq", [F, r], i8, kind="ExternalOutput")

        Copy = mybir.ActivationFunctionType.Copy

        with TileContext(nc) as tc:
            with (
                tc.tile_pool(name="consts", bufs=1) as cpool,
                tc.tile_pool(name="pin", bufs=6) as ipool,
                tc.tile_pool(name="pf16", bufs=4) as fpool,
                tc.tile_pool(name="po", bufs=4) as opool,
                tc.tile_pool(name="ps", bufs=2, space="PSUM") as pspool,
            ):
                # weights ride the scalar engine's HWDGE FIFO so they can't
                # head-block the sync engine's data queue
                w_sb = cpool.tile([F, F], fp16)
                nc.scalar.dma_start(out=w_sb[:], in_=wq[:, :])

                o = 0
                for csz in _io_chunks(r):
                    x8 = ipool.tile([F, CH_IO], i8, tag="x8")
                    nc.sync.dma_start(
                        out=x8[:, :csz], in_=xq[:, o:o + csz]
                    )
                    x16 = fpool.tile([F, CH_IO], fp16, tag="x16")
                    y8 = opool.tile([F, CH_IO], i8, tag="y8")
                    for po in range(0, csz, CH_PS):
                        psz = min(CH_PS, csz - po)
                        # conv per psum tile so matmuls start earlier
                        nc.vector.tensor_copy(
                            x16[:, po:po + psz], x8[:, po:po + psz]
                        )
                        ps = pspool.tile([F, CH_PS], f32, tag="ps")
                        for mo in range(0, psz, MM_N):
                            nc.tensor.matmul(
                                out=ps[:, mo:mo + MM_N],
                                lhsT=w_sb[:],
                                rhs=x16[:, po + mo:po + mo + MM_N],
                                start=True, stop=True,
                            )
                        # evac split: ACT head + DVE tail finish together
                        dcols = DVE_EVAC * psz // CH_PS
                        acols = psz - dcols
                        nc.scalar.activation(
                            y8[:, po:po + acols], ps[:, 0:acols], Copy,
                            bias=0.0, scale=1.0,
                        )
                        nc.vector.tensor_copy(
                            y8[:, po + acols:po + psz], ps[:, acols:psz]
                        )
                    nc.sync.dma_start(
                        out=yq[:, o:o + csz], in_=y8[:, :csz]
                    )
                    o += csz
    finally:
        TileContext._drain_and_barrier = _orig_dab

    # Drop redundant Ldweights: every matmul reloads the same stationary
    # weights; only the first load (and any Ldweights carrying semaphore
    # waits, which must be preserved for sync correctness) are kept.
    # Weights stay resident in the PE array across matmuls.
    first_kept = False
    for f in nc.m.functions:
        for b in f.blocks:
            insts = list(b.instructions)
            keep = []
            changed = False
            for inst in insts:
                if str(inst.opcode) == "Ldweights":
                    si = inst.sync_info
                    has_sync = si is not None and (
                        len(si.on_wait) > 0 or len(si.on_update) > 0
                    )
                    if first_kept and not has_sync:
                        changed = True
                        continue
                    first_kept = True
                keep.append(inst)
            if changed:
                b.instructions = keep

    nc.compile()
    return nc


_NC_CACHE = {}


def _get_nc(key=None):
    if key not in _NC_CACHE:
        _NC_CACHE[key] = _build_nc()
    return _NC_CACHE[key]


def compose_matrix(angles, indices_in, idx_out):
    """Compose the butterfly layers into one [F, F] matrix (float64)."""
    angles = np.asarray(angles, dtype=np.float64)
    ii = np.asarray(indices_in).reshape(-1, 2)
    io = np.asarray(idx_out).reshape(-1, 2)
    M = np.eye(F, dtype=np.float64)
    for l in range(angles.shape[0]):
        c = np.cos(angles[l])
        s = np.sin(angles[l])
        A = np.eye(F, dtype=np.float64)
        A[io[:, 0], :] = 0.0
        A[io[:, 1], :] = 0.0
        A[io[:, 0], ii[:, 0]] = c
        A[io[:, 0], ii[:, 1]] = -s
        A[io[:, 1], ii[:, 0]] = s
        A[io[:, 1], ii[:, 1]] = c
        M = A @ M
    return M


def _output_bounds(M, data, amax, indices_in, idx_out):
    """Per-output-feature sup bound on |y_m| (float64).

    When M is pair-block structured (idx_out == indices_in composes each
    pair's rotations), |y| for both outputs of pair p is bounded by the
    pair's max batch radius (rotation-invariant, exact). Otherwise fall
    back to the Hoelder bound sum_k |M[m,k]| amax_k.
    """
    ii = np.asarray(indices_in).reshape(-1, 2)
    io = np.asarray(idx_out).reshape(-1, 2)
    ia, ib = ii[:, 0], ii[:, 1]
    oa, ob = io[:, 0], io[:, 1]
    mask = np.zeros((F, F), dtype=bool)
    mask[oa, ia] = mask[oa, ib] = mask[ob, ia] = mask[ob, ib] = True
    bound = np.abs(M) @ amax  # Hoelder, always valid
    if not np.any(M[~mask] != 0.0):
        a = data[:, ia].astype(np.float64)
        b = data[:, ib].astype(np.float64)
        radius = np.sqrt(np.max(a * a + b * b, axis=0))  # [64]
        pb = np.empty(F, dtype=np.float64)
        pb[oa] = radius
        pb[ob] = radius
        bound = np.minimum(bound, pb)
    return bound


def _run(data, angles, indices_in, idx_out, trace=False):
    from concourse.bass_utils import run_bass_kernel_spmd

    data = np.asarray(data)
    assert data.shape == (B, F) and data.dtype == np.float32, (
        f"unexpected data {data.shape} {data.dtype}"
    )
    M = compose_matrix(angles, indices_in, idx_out)

    amax = np.abs(data).max(axis=0).astype(np.float64)  # [F]
    s_in = np.maximum(amax, 1e-30) / 127.0
    bound = _output_bounds(M, data, amax, indices_in, idx_out)
    s_out = np.maximum(bound, 1e-30) * 1.02 / 127.0

    # lhsT[k, m] = M[m, k] * s_in[k] / s_out[m]
    lhsT = (M.T * s_in[:, None] / s_out[None, :]).astype(np.float16)
    lhsT = np.ascontiguousarray(lhsT)

    # quantize: x_q = rint(x / s_in), feature-major per core
    xq_all = np.rint(data * (1.0 / s_in).astype(np.float32)[None, :])
    xq_all = np.clip(xq_all, -127, 127).astype(np.int8)

    in_maps = []
    for i in range(NUM_CORES):
        r0 = i * R
        xq_i = np.ascontiguousarray(xq_all[r0:r0 + R, :].T)  # [F, R]
        in_maps.append({"xq": xq_i, "wq": lhsT})

    nc = _get_nc()
    res = run_bass_kernel_spmd(
        nc, in_maps, core_ids=list(range(NUM_CORES)), trace=trace
    )

    s_out32 = s_out.astype(np.float32)
    out = np.empty((B, F), dtype=np.float32)
    for i in range(NUM_CORES):
        r0 = i * R
        yq_i = res.results[i]["yq"]  # [F, R] int8
        out[r0:r0 + R, :] = yq_i.T.astype(np.float32) * s_out32[None, :]
    return out, res


def kernel(data, angles, indices_in, idx_out):
    out, _ = _run(data, angles, indices_in, idx_out, trace=False)
    return out
